# revision 37
# baseline (speedup 1.0000x reference)
"""Trainium2 Bass kernel for nn_MessagePassing (vertical message passing).

Computation (per batch element b):
    y[0] = x[0]
    y[i] = x[i] + relu(conv1d_same(y[i-1], W))   for i = 1..H-1
with x (H, W, C) = (128, 256, 128) fp32, W (K, Cin, Cout) = (9, 128, 128).

Sharding: batch B=8 across the 8 NeuronCores (data parallel, no
communication). Each core runs the sequential H recurrence for one batch
element.

Per-core design (transposed recurrence, fp16):
  - The recurrent state yT lives in SBUF transposed+padded as fp16
    (C=128 partitions, W+8 cols). x is passed from the host already
    transposed to (H, C, W) fp16, so the DVE writes the new state directly:
        yT_new[:, w] = max(psum_conv[:, w], 0) + xT_i[:, w]
    and the serial recurrence has NO transposes and only two cross-engine
    hops per step (PE conv -> DVE relu+add -> PE conv).
  - Conv: 2 w-tiles x 9 taps of accumulating matmuls, stationary =
    W16[:, k, :], moving = state slice (fp16: 1 cycle/row at any width).
    Taps are emitted halo-safe-first so the next step's first matmuls only
    depend on the state tile whose relu+add finished first; the PE never
    goes idle, which also keeps the PE at its ramped (full) clock.
  - Output rows are transposed back to natural (w, c) layout OFF the
    critical path: PE fp16 transposes of the state halves (deferred past
    the next step's halo-free conv block so they never stall the PE),
    then ACT upcasts to fp32 staging and the row DMAs out.
"""

import numpy as np

B, H, W_DIM, C, K = 8, 128, 256, 128, 9
PAD = 4
WBUF = W_DIM + 2 * PAD  # 264
P = 128

# w-split: tile0 = w in [0, S0), tile1 = w in [S0, W_DIM). Asymmetric:
# tile0 (computed last) is smaller, so its relu+add lands earlier and the
# next step's halo taps have more conv work to hide the latency behind.
S0 = 96
# boundary width: the last BW w-columns of tile0 get their own tiny
# relu+add so the next step's tile1 halo taps (which only read w >= S0-4)
# gate on a ~60ns DVE op instead of the full tile0 op
BW = 4
# halo-safe-first tap orders (tile1 taps k>=4 read only tile1's state
# columns; tile0 taps k<=4 read only tile0's)
TAPS1 = [4, 5, 6, 7, 8, 3, 2, 1, 0]
TAPS0 = [4, 3, 2, 1, 0, 5, 6, 7, 8]

_NC_CACHE = {}


def _emit_body(nc, mybir, f32, f16, x_d, o_d, pools, w16, ident16):
    (xin_pool, state_pool, stage_pool, pconv_pool, pout_pool) = pools
    S1W = W_DIM - S0

    yT = []
    for j in range(2):
        t = state_pool.tile([P, WBUF], f16, tag=f"yT{j}", name=f"yT{j}")
        nc.vector.memset(t[:], 0.0)
        yT.append(t)

    # x rows arrive in batches of XB per DMA dispatch (SP queue relief)
    XB = 4
    x_tiles = {}

    def load_x_batch(j):
        if j * XB >= H:
            return
        n = min(XB, H - j * XB)
        t = xin_pool.tile([P, XB, W_DIM], f16, tag="xt", name=f"xt{j}")
        nc.sync.dma_start(
            t[:, 0:n, :], x_d[j * XB : j * XB + n].rearrange("h c w -> c h w")
        )
        x_tiles[j] = t

    def x_row(i):
        return x_tiles[i // XB][:, i % XB, :]

    PREFETCH_B = 2
    for j in range(PREFETCH_B):
        load_x_batch(j)

    # --- output path helpers (all off the recurrence critical path) ---
    def emit_oxp(i, t, src_state):
        # transpose state half t of row i -> natural (w, c) fp16 PSUM
        po = pout_pool.tile([P, P], f16, tag=f"po{t}", name=f"po{t}_{i}")
        nc.tensor.matmul(
            po[:], src_state[:, PAD + t * P : PAD + (t + 1) * P], ident16[:],
            is_transpose=True,
        )
        return po

    def emit_out_half(i, t, po, st):
        # upcast fp16 PSUM -> fp32 staging on ACT
        nc.scalar.copy(st[:, t, :], po[:])

    # PE pstate warmup: dummy transposes on constant data fill the PE while
    # the prologue DMAs land, so the tensor engine's clock is already
    # ramping when the first real conv issues (it only reaches full rate
    # after ~3us of continuous busy)
    for r in range(16):
        po = pout_pool.tile([P, P], f16, tag=f"po{r % 2}", name=f"warm{r}")
        nc.tensor.matmul(po[:], ident16[:], ident16[:], is_transpose=True)

    # y_0 = x_0: state loads straight from DRAM (already transposed fp16).
    # Issued on the gpsimd (Pool) queue: SP is busy with w16 + x prefetches
    # and ACT is blocked by LoadActFuncSet at trip start.
    nc.gpsimd.dma_start(yT[0][:, PAD : PAD + W_DIM], x_d[0])

    # row 0 output: transpose back on PE, upcast, DMA (prologue)
    st0 = stage_pool.tile([P, 2, C], f32, tag="stage", name="st_r0")
    for t in (1, 0):
        po = emit_oxp(0, t, yT[0])
        emit_out_half(0, t, po, st0)
    nc.sync.dma_start(o_d[0].rearrange("(t w) c -> w t c", t=2), st0[:])

    # deferred output work from the previous step:
    #   [(emit_fn, ...)] executed inside the next step's PE stream
    pending = None  # (i_prev, po1_prev)

    for i in range(1, H):
        a, b = (i - 1) % 2, i % 2

        # --- conv tile1 (halo-free taps first) ---
        pc1 = pconv_pool.tile([P, S1W], f32, tag="pn1", name=f"pn1_{i}")
        for j, k in enumerate(TAPS1):
            nc.tensor.matmul(
                pc1[:], w16[:, k, :], yT[a][:, S0 + k : S0 + k + S1W],
                start=(j == 0), stop=(j == K - 1),
            )

        # deferred from step i-1: transpose + upcast + DMA of row i-1's
        # tile0 (its stt0 finished while our tile1 block was streaming)
        if pending is not None:
            ip, po1p, stp = pending
            po0p = emit_oxp(ip, 0, yT[a])
            emit_out_half(ip, 0, po0p, stp)
            nc.sync.dma_start(
                o_d[ip].rearrange("(t w) c -> w t c", t=2), stp[:]
            )
            pending = None

        # relu + residual for tile1 -> state (fp16, direct)
        nc.vector.scalar_tensor_tensor(
            yT[b][:, PAD + S0 : PAD + W_DIM],
            pc1[:], 0.0, x_row(i)[:, S0:W_DIM],
            op0=mybir.AluOpType.max, op1=mybir.AluOpType.add,
        )

        # --- conv tile0 ---
        pc0 = pconv_pool.tile([P, S0], f32, tag="pn0", name=f"pn0_{i}")
        for j, k in enumerate(TAPS0):
            nc.tensor.matmul(
                pc0[:], w16[:, k, :], yT[a][:, k : k + S0],
                start=(j == 0), stop=(j == K - 1),
            )
        # boundary columns first (gates the next step's tile1 halo taps)
        nc.vector.scalar_tensor_tensor(
            yT[b][:, PAD + S0 - BW : PAD + S0],
            pc0[:, S0 - BW : S0], 0.0, x_row(i)[:, S0 - BW : S0],
            op0=mybir.AluOpType.max, op1=mybir.AluOpType.add,
        )
        nc.vector.scalar_tensor_tensor(
            yT[b][:, PAD : PAD + S0 - BW],
            pc0[:, 0 : S0 - BW], 0.0, x_row(i)[:, 0 : S0 - BW],
            op0=mybir.AluOpType.max, op1=mybir.AluOpType.add,
        )

        # tile1's output transpose: stt1 finished during the tile0 block
        st = stage_pool.tile([P, 2, C], f32, tag="stage", name=f"st{i}")
        po1 = emit_oxp(i, 1, yT[b])
        emit_out_half(i, 1, po1, st)
        pending = (i, po1, st)

        if i % XB == 0:
            load_x_batch(i // XB + PREFETCH_B - 1)
            x_tiles.pop(i // XB - 1, None)

    # epilogue: flush the last deferred row
    ip, po1p, stp = pending
    po0p = emit_oxp(ip, 0, yT[(H - 1) % 2])
    emit_out_half(ip, 0, po0p, stp)
    nc.sync.dma_start(o_d[ip].rearrange("(t w) c -> w t c", t=2), stp[:])


def _build_nc(reps=1):
    """Build the kernel module. reps>1 wraps the whole computation in a
    hardware loop that repeats it (identical work each trip) -- used only to
    measure device execution time above the dispatch-noise floor."""
    import contextlib

    import concourse.tile as tile
    from concourse import bacc, mybir
    from concourse.masks import make_identity

    f32 = mybir.dt.float32
    f16 = mybir.dt.float16

    nc = bacc.Bacc("TRN2", target_bir_lowering=False, debug=False, num_devices=B)
    # x arrives pre-transposed (C, W) per row, fp16; W pre-arranged
    # (Cin, K, Cout) fp16 (host-side layout prep)
    x_d = nc.dram_tensor("x", [H, C, W_DIM], f16, kind="ExternalInput").ap()
    w_d = nc.dram_tensor("w", [C, K, C], f16, kind="ExternalInput").ap()
    o_d = nc.dram_tensor("out", [H, W_DIM, C], f32, kind="ExternalOutput").ap()

    with tile.TileContext(nc) as tc:
        with (
            tc.tile_pool(name="xin", bufs=6) as xin_pool,
            tc.tile_pool(name="state", bufs=1) as state_pool,
            tc.tile_pool(name="stage", bufs=4) as stage_pool,
            tc.tile_pool(name="const", bufs=1) as const_pool,
            tc.tile_pool(name="pconv", bufs=2, space="PSUM") as pconv_pool,
            tc.tile_pool(name="pout", bufs=2, space="PSUM") as pout_pool,
        ):
            # weights -> SBUF (ci partitions, K, co) fp16, single clean DMA
            # (first on the SP queue: it gates the first conv)
            w16 = const_pool.tile([P, K, C], f16, name="w16")
            nc.sync.dma_start(w16[:], w_d)

            ident = const_pool.tile([P, P], f32, name="ident")
            make_identity(nc, ident[:])
            ident16 = const_pool.tile([P, P], f16, name="ident16")
            nc.vector.tensor_copy(ident16[:], ident[:])

            pools = (xin_pool, state_pool, stage_pool, pconv_pool, pout_pool)
            rep_ctx = tc.For_i(0, reps, 1) if reps > 1 else contextlib.nullcontext()
            with rep_ctx:
                _emit_body(nc, mybir, f32, f16, x_d, o_d, pools, w16, ident16)

    nc.compile()
    return nc


def _get_nc():
    if "nc" not in _NC_CACHE:
        _NC_CACHE["nc"] = _build_nc()
    return _NC_CACHE["nc"]


def _prep_x(xb):
    # (H, W, C) fp32 -> (H, C, W) fp16 host-side layout prep
    return np.ascontiguousarray(xb.transpose(0, 2, 1)).astype(np.float16)


def _prep_w(W):
    # (K, Cin, Cout) fp32 -> (Cin, K, Cout) fp16 host-side layout prep
    return np.ascontiguousarray(W.transpose(1, 0, 2)).astype(np.float16)


def kernel(x, W):
    """Full-input entry point: shard batch B across the 8 NeuronCores (data
    parallel), run the Bass kernel, gather per-core outputs."""
    from concourse.bass_utils import run_bass_kernel_spmd

    x = np.asarray(x, dtype=np.float32)
    W = np.asarray(W, dtype=np.float32)
    assert x.shape == (B, H, W_DIM, C), x.shape
    assert W.shape == (K, C, C), W.shape

    nc = _get_nc()
    w16 = _prep_w(W)
    in_maps = [{"x": _prep_x(x[b]), "w": w16} for b in range(B)]
    res = run_bass_kernel_spmd(nc, in_maps, core_ids=list(range(B)))
    return np.stack([np.asarray(res.results[b]["out"]) for b in range(B)], axis=0)


# revision 38
# speedup vs baseline: 1.0040x; 1.0040x over previous
"""Trainium2 Bass kernel for nn_MessagePassing (vertical message passing).

Computation (per batch element b):
    y[0] = x[0]
    y[i] = x[i] + relu(conv1d_same(y[i-1], W))   for i = 1..H-1
with x (H, W, C) = (128, 256, 128) fp32, W (K, Cin, Cout) = (9, 128, 128).

Sharding: batch B=8 across the 8 NeuronCores (data parallel, no
communication). Each core runs the sequential H recurrence for one batch
element.

Per-core design (transposed recurrence, fp16):
  - The recurrent state yT lives in SBUF transposed+padded as fp16
    (C=128 partitions, W+8 cols). x is passed from the host already
    transposed to (H, C, W) fp16, so the DVE writes the new state directly:
        yT_new[:, w] = max(psum_conv[:, w], 0) + xT_i[:, w]
    and the serial recurrence has NO transposes and only two cross-engine
    hops per step (PE conv -> DVE relu+add -> PE conv).
  - Conv: 2 w-tiles x 9 taps of accumulating matmuls, stationary =
    W16[:, k, :], moving = state slice (fp16: 1 cycle/row at any width).
    Taps are emitted halo-safe-first so the next step's first matmuls only
    depend on the state tile whose relu+add finished first; the PE never
    goes idle, which also keeps the PE at its ramped (full) clock.
  - Output rows are transposed back to natural (w, c) layout OFF the
    critical path: PE fp16 transposes of the state halves (deferred past
    the next step's halo-free conv block so they never stall the PE),
    then ACT upcasts to fp32 staging and the row DMAs out.
"""

import numpy as np

B, H, W_DIM, C, K = 8, 128, 256, 128, 9
PAD = 4
WBUF = W_DIM + 2 * PAD  # 264
P = 128

# w-split: tile0 = w in [0, S0), tile1 = w in [S0, W_DIM). Asymmetric:
# tile0 (computed last) is smaller, so its relu+add lands earlier and the
# next step's halo taps have more conv work to hide the latency behind.
S0 = 96
# boundary width: the last BW w-columns of tile0 get their own tiny
# relu+add so the next step's tile1 halo taps (which only read w >= S0-4)
# gate on a ~60ns DVE op instead of the full tile0 op
BW = 4
# halo-safe-first tap orders (tile1 taps k>=4 read only tile1's state
# columns; tile0 taps k<=4 read only tile0's)
TAPS1 = [4, 5, 6, 7, 8, 3, 2, 1, 0]
TAPS0 = [4, 3, 2, 1, 0, 5, 6, 7, 8]

_NC_CACHE = {}


def _emit_body(nc, mybir, f32, f16, x_d, o_d, pools, w16, ident16):
    (xin_pool, state_pool, stage_pool, pconv_pool, pout_pool) = pools
    S1W = W_DIM - S0

    yT = []
    for j in range(2):
        t = state_pool.tile([P, WBUF], f16, tag=f"yT{j}", name=f"yT{j}")
        nc.vector.memset(t[:], 0.0)
        yT.append(t)

    # x rows arrive in batches of XB per DMA dispatch (SP queue relief)
    XB = 4
    x_tiles = {}

    def load_x_batch(j):
        if j * XB >= H:
            return
        n = min(XB, H - j * XB)
        t = xin_pool.tile([P, XB, W_DIM], f16, tag="xt", name=f"xt{j}")
        nc.sync.dma_start(
            t[:, 0:n, :], x_d[j * XB : j * XB + n].rearrange("h c w -> c h w")
        )
        x_tiles[j] = t

    def x_row(i):
        return x_tiles[i // XB][:, i % XB, :]

    PREFETCH_B = 2
    for j in range(PREFETCH_B):
        load_x_batch(j)

    # --- output path helpers (all off the recurrence critical path) ---
    def emit_oxp(i, t, src_state):
        # transpose state half t of row i -> natural (w, c) fp16 PSUM
        po = pout_pool.tile([P, P], f16, tag=f"po{t}", name=f"po{t}_{i}")
        nc.tensor.matmul(
            po[:], src_state[:, PAD + t * P : PAD + (t + 1) * P], ident16[:],
            is_transpose=True,
        )
        return po

    def emit_out_half(i, t, po, st):
        # upcast fp16 PSUM -> fp32 staging on ACT
        nc.scalar.copy(st[:, t, :], po[:])

    # PE pstate warmup: dummy transposes on constant data fill the PE while
    # the prologue DMAs land, so the tensor engine's clock is already
    # ramping when the first real conv issues (it only reaches full rate
    # after ~3us of continuous busy)
    for r in range(16):
        po = pout_pool.tile([P, P], f16, tag=f"po{r % 2}", name=f"warm{r}")
        nc.tensor.matmul(po[:], ident16[:], ident16[:], is_transpose=True)

    # y_0 = x_0: state loads straight from DRAM (already transposed fp16).
    # Issued on the ACT queue so it runs parallel to w16 + x prefetches on
    # SP (this DMA + w16 gate the first conv).
    nc.scalar.dma_start(yT[0][:, PAD : PAD + W_DIM], x_d[0])

    # row 0 output: transpose back on PE, upcast, DMA (prologue)
    st0 = stage_pool.tile([P, 2, C], f32, tag="stage", name="st_r0")
    for t in (1, 0):
        po = emit_oxp(0, t, yT[0])
        emit_out_half(0, t, po, st0)
    nc.sync.dma_start(o_d[0].rearrange("(t w) c -> w t c", t=2), st0[:])

    # deferred output work from the previous step:
    #   [(emit_fn, ...)] executed inside the next step's PE stream
    pending = None  # (i_prev, po1_prev)

    for i in range(1, H):
        a, b = (i - 1) % 2, i % 2

        # --- conv tile1 (halo-free taps first) ---
        pc1 = pconv_pool.tile([P, S1W], f32, tag="pn1", name=f"pn1_{i}")
        for j, k in enumerate(TAPS1):
            nc.tensor.matmul(
                pc1[:], w16[:, k, :], yT[a][:, S0 + k : S0 + k + S1W],
                start=(j == 0), stop=(j == K - 1),
            )

        # deferred from step i-1: transpose + upcast + DMA of row i-1's
        # tile0 (its stt0 finished while our tile1 block was streaming)
        if pending is not None:
            ip, po1p, stp = pending
            po0p = emit_oxp(ip, 0, yT[a])
            emit_out_half(ip, 0, po0p, stp)
            nc.sync.dma_start(
                o_d[ip].rearrange("(t w) c -> w t c", t=2), stp[:]
            )
            pending = None

        # relu + residual for tile1 -> state (fp16, direct)
        nc.vector.scalar_tensor_tensor(
            yT[b][:, PAD + S0 : PAD + W_DIM],
            pc1[:], 0.0, x_row(i)[:, S0:W_DIM],
            op0=mybir.AluOpType.max, op1=mybir.AluOpType.add,
        )

        # --- conv tile0 ---
        pc0 = pconv_pool.tile([P, S0], f32, tag="pn0", name=f"pn0_{i}")
        for j, k in enumerate(TAPS0):
            nc.tensor.matmul(
                pc0[:], w16[:, k, :], yT[a][:, k : k + S0],
                start=(j == 0), stop=(j == K - 1),
            )
        # boundary columns first (gates the next step's tile1 halo taps)
        nc.vector.scalar_tensor_tensor(
            yT[b][:, PAD + S0 - BW : PAD + S0],
            pc0[:, S0 - BW : S0], 0.0, x_row(i)[:, S0 - BW : S0],
            op0=mybir.AluOpType.max, op1=mybir.AluOpType.add,
        )
        nc.vector.scalar_tensor_tensor(
            yT[b][:, PAD : PAD + S0 - BW],
            pc0[:, 0 : S0 - BW], 0.0, x_row(i)[:, 0 : S0 - BW],
            op0=mybir.AluOpType.max, op1=mybir.AluOpType.add,
        )

        # tile1's output transpose: stt1 finished during the tile0 block
        st = stage_pool.tile([P, 2, C], f32, tag="stage", name=f"st{i}")
        po1 = emit_oxp(i, 1, yT[b])
        emit_out_half(i, 1, po1, st)
        pending = (i, po1, st)

        if i % XB == 0:
            load_x_batch(i // XB + PREFETCH_B - 1)
            x_tiles.pop(i // XB - 1, None)

    # epilogue: flush the last deferred row
    ip, po1p, stp = pending
    po0p = emit_oxp(ip, 0, yT[(H - 1) % 2])
    emit_out_half(ip, 0, po0p, stp)
    nc.sync.dma_start(o_d[ip].rearrange("(t w) c -> w t c", t=2), stp[:])


def _build_nc(reps=1):
    """Build the kernel module. reps>1 wraps the whole computation in a
    hardware loop that repeats it (identical work each trip) -- used only to
    measure device execution time above the dispatch-noise floor."""
    import contextlib

    import concourse.tile as tile
    from concourse import bacc, mybir
    from concourse.masks import make_identity

    f32 = mybir.dt.float32
    f16 = mybir.dt.float16

    nc = bacc.Bacc("TRN2", target_bir_lowering=False, debug=False, num_devices=B)
    # x arrives pre-transposed (C, W) per row, fp16; W pre-arranged
    # (Cin, K, Cout) fp16 (host-side layout prep)
    x_d = nc.dram_tensor("x", [H, C, W_DIM], f16, kind="ExternalInput").ap()
    w_d = nc.dram_tensor("w", [C, K, C], f16, kind="ExternalInput").ap()
    o_d = nc.dram_tensor("out", [H, W_DIM, C], f32, kind="ExternalOutput").ap()

    with tile.TileContext(nc) as tc:
        with (
            tc.tile_pool(name="xin", bufs=6) as xin_pool,
            tc.tile_pool(name="state", bufs=1) as state_pool,
            tc.tile_pool(name="stage", bufs=4) as stage_pool,
            tc.tile_pool(name="const", bufs=1) as const_pool,
            tc.tile_pool(name="pconv", bufs=2, space="PSUM") as pconv_pool,
            tc.tile_pool(name="pout", bufs=2, space="PSUM") as pout_pool,
        ):
            # weights -> SBUF (ci partitions, K, co) fp16, single clean DMA
            # (first on the SP queue: it gates the first conv)
            w16 = const_pool.tile([P, K, C], f16, name="w16")
            nc.sync.dma_start(w16[:], w_d)

            ident = const_pool.tile([P, P], f32, name="ident")
            make_identity(nc, ident[:])
            ident16 = const_pool.tile([P, P], f16, name="ident16")
            nc.vector.tensor_copy(ident16[:], ident[:])

            pools = (xin_pool, state_pool, stage_pool, pconv_pool, pout_pool)
            rep_ctx = tc.For_i(0, reps, 1) if reps > 1 else contextlib.nullcontext()
            with rep_ctx:
                _emit_body(nc, mybir, f32, f16, x_d, o_d, pools, w16, ident16)

    nc.compile()
    return nc


def _get_nc():
    if "nc" not in _NC_CACHE:
        _NC_CACHE["nc"] = _build_nc()
    return _NC_CACHE["nc"]


def _prep_x(xb):
    # (H, W, C) fp32 -> (H, C, W) fp16 host-side layout prep
    return np.ascontiguousarray(xb.transpose(0, 2, 1)).astype(np.float16)


def _prep_w(W):
    # (K, Cin, Cout) fp32 -> (Cin, K, Cout) fp16 host-side layout prep
    return np.ascontiguousarray(W.transpose(1, 0, 2)).astype(np.float16)


def kernel(x, W):
    """Full-input entry point: shard batch B across the 8 NeuronCores (data
    parallel), run the Bass kernel, gather per-core outputs."""
    from concourse.bass_utils import run_bass_kernel_spmd

    x = np.asarray(x, dtype=np.float32)
    W = np.asarray(W, dtype=np.float32)
    assert x.shape == (B, H, W_DIM, C), x.shape
    assert W.shape == (K, C, C), W.shape

    nc = _get_nc()
    w16 = _prep_w(W)
    in_maps = [{"x": _prep_x(x[b]), "w": w16} for b in range(B)]
    res = run_bass_kernel_spmd(nc, in_maps, core_ids=list(range(B)))
    return np.stack([np.asarray(res.results[b]["out"]) for b in range(B)], axis=0)


# revision 39
# speedup vs baseline: 1.0483x; 1.0441x over previous
"""Trainium2 Bass kernel for nn_MessagePassing (vertical message passing).

Computation (per batch element b):
    y[0] = x[0]
    y[i] = x[i] + relu(conv1d_same(y[i-1], W))   for i = 1..H-1
with x (H, W, C) = (128, 256, 128) fp32, W (K, Cin, Cout) = (9, 128, 128).

Sharding: batch B=8 across the 8 NeuronCores (data parallel, no
communication). Each core runs the sequential H recurrence for one batch
element.

Per-core design (transposed recurrence, fp16):
  - The recurrent state yT lives in SBUF transposed+padded as fp16
    (C=128 partitions, W+8 cols). x is passed from the host already
    transposed to (H, C, W) fp16, so the DVE writes the new state directly:
        yT_new[:, w] = max(psum_conv[:, w], 0) + xT_i[:, w]
    and the serial recurrence has NO transposes and only two cross-engine
    hops per step (PE conv -> DVE relu+add -> PE conv).
  - Conv: 2 w-tiles x 9 taps of accumulating matmuls, stationary =
    W16[:, k, :], moving = state slice (fp16: 1 cycle/row at any width).
    Taps are emitted halo-safe-first so the next step's first matmuls only
    depend on the state tile whose relu+add finished first; the PE never
    goes idle, which also keeps the PE at its ramped (full) clock.
  - Output rows are transposed back to natural (w, c) layout OFF the
    critical path: PE fp16 transposes of the state halves (deferred past
    the next step's halo-free conv block so they never stall the PE),
    then ACT upcasts to fp32 staging and the row DMAs out.
"""

import numpy as np

B, H, W_DIM, C, K = 8, 128, 256, 128, 9
PAD = 4
WBUF = W_DIM + 2 * PAD  # 264
P = 128

# w-split: tile0 = w in [0, S0), tile1 = w in [S0, W_DIM). Asymmetric:
# tile0 (computed last) is smaller, so its relu+add lands earlier and the
# next step's halo taps have more conv work to hide the latency behind.
S0 = 96
# boundary width: the last BW w-columns of tile0 get their own tiny
# relu+add so the next step's tile1 halo taps (which only read w >= S0-4)
# gate on a ~60ns DVE op instead of the full tile0 op
BW = 4
# halo-safe-first tap orders (tile1 taps k>=4 read only tile1's state
# columns; tile0 taps k<=4 read only tile0's)
TAPS1 = [4, 5, 6, 7, 8, 3, 2, 1, 0]
TAPS0 = [4, 3, 2, 1, 0, 5, 6, 7, 8]

_NC_CACHE = {}


def _emit_body(nc, mybir, f32, f16, x_d, o_d, pools, w16, ident16):
    (xin_pool, state_pool, stage_pool, pconv_pool, pout_pool) = pools
    S1W = W_DIM - S0

    yT = []
    for j in range(2):
        t = state_pool.tile([P, WBUF], f16, tag=f"yT{j}", name=f"yT{j}")
        nc.vector.memset(t[:], 0.0)
        yT.append(t)

    # x rows arrive in batches of XB per DMA dispatch (SP queue relief)
    XB = 4
    x_tiles = {}

    def load_x_batch(j):
        if j * XB >= H:
            return
        n = min(XB, H - j * XB)
        t = xin_pool.tile([P, XB, W_DIM], f16, tag="xt", name=f"xt{j}")
        nc.sync.dma_start(
            t[:, 0:n, :], x_d[j * XB : j * XB + n].rearrange("h c w -> c h w")
        )
        x_tiles[j] = t

    def x_row(i):
        return x_tiles[i // XB][:, i % XB, :]

    PREFETCH_B = 2
    for j in range(PREFETCH_B):
        load_x_batch(j)

    # --- output path helpers (all off the recurrence critical path) ---
    def emit_oxp(i, t, src_state):
        # transpose state half t of row i -> natural (w, c) fp16 PSUM
        po = pout_pool.tile([P, P], f16, tag=f"po{t}", name=f"po{t}_{i}")
        nc.tensor.matmul(
            po[:], src_state[:, PAD + t * P : PAD + (t + 1) * P], ident16[:],
            is_transpose=True,
        )
        return po

    def emit_out_half(i, t, po, st):
        # upcast fp16 PSUM -> fp32 staging on ACT
        nc.scalar.copy(st[:, t, :], po[:])

    # y_0 = x_0: state loads straight from DRAM (already transposed fp16).
    # Issued on the ACT queue so it runs parallel to the x prefetches on SP
    # (this DMA + w16 gate the first conv).
    nc.scalar.dma_start(yT[0][:, PAD : PAD + W_DIM], x_d[0])

    # row 0 output: transpose back on PE, upcast, DMA (prologue)
    st0 = stage_pool.tile([P, 2, C], f32, tag="stage", name="st_r0")
    for t in (1, 0):
        po = emit_oxp(0, t, yT[0])
        emit_out_half(0, t, po, st0)
    nc.sync.dma_start(o_d[0].rearrange("(t w) c -> w t c", t=2), st0[:])

    # deferred output work from the previous step:
    #   [(emit_fn, ...)] executed inside the next step's PE stream
    pending = None  # (i_prev, po1_prev)

    for i in range(1, H):
        a, b = (i - 1) % 2, i % 2

        # --- conv tile1 (halo-free taps first) ---
        pc1 = pconv_pool.tile([P, S1W], f32, tag="pn1", name=f"pn1_{i}")
        for j, k in enumerate(TAPS1):
            nc.tensor.matmul(
                pc1[:], w16[:, k, :], yT[a][:, S0 + k : S0 + k + S1W],
                start=(j == 0), stop=(j == K - 1),
            )

        # deferred from step i-1: transpose + upcast + DMA of row i-1's
        # tile0 (its stt0 finished while our tile1 block was streaming)
        if pending is not None:
            ip, po1p, stp = pending
            po0p = emit_oxp(ip, 0, yT[a])
            emit_out_half(ip, 0, po0p, stp)
            nc.sync.dma_start(
                o_d[ip].rearrange("(t w) c -> w t c", t=2), stp[:]
            )
            pending = None

        # relu + residual for tile1 -> state (fp16, direct)
        nc.vector.scalar_tensor_tensor(
            yT[b][:, PAD + S0 : PAD + W_DIM],
            pc1[:], 0.0, x_row(i)[:, S0:W_DIM],
            op0=mybir.AluOpType.max, op1=mybir.AluOpType.add,
        )

        # --- conv tile0 ---
        pc0 = pconv_pool.tile([P, S0], f32, tag="pn0", name=f"pn0_{i}")
        for j, k in enumerate(TAPS0):
            nc.tensor.matmul(
                pc0[:], w16[:, k, :], yT[a][:, k : k + S0],
                start=(j == 0), stop=(j == K - 1),
            )
        # boundary columns first (gates the next step's tile1 halo taps)
        nc.vector.scalar_tensor_tensor(
            yT[b][:, PAD + S0 - BW : PAD + S0],
            pc0[:, S0 - BW : S0], 0.0, x_row(i)[:, S0 - BW : S0],
            op0=mybir.AluOpType.max, op1=mybir.AluOpType.add,
        )
        nc.vector.scalar_tensor_tensor(
            yT[b][:, PAD : PAD + S0 - BW],
            pc0[:, 0 : S0 - BW], 0.0, x_row(i)[:, 0 : S0 - BW],
            op0=mybir.AluOpType.max, op1=mybir.AluOpType.add,
        )

        # tile1's output transpose: stt1 finished during the tile0 block
        st = stage_pool.tile([P, 2, C], f32, tag="stage", name=f"st{i}")
        po1 = emit_oxp(i, 1, yT[b])
        emit_out_half(i, 1, po1, st)
        pending = (i, po1, st)

        if i % XB == 0:
            load_x_batch(i // XB + PREFETCH_B - 1)
            x_tiles.pop(i // XB - 1, None)

    # epilogue: flush the last deferred row
    ip, po1p, stp = pending
    po0p = emit_oxp(ip, 0, yT[(H - 1) % 2])
    emit_out_half(ip, 0, po0p, stp)
    nc.sync.dma_start(o_d[ip].rearrange("(t w) c -> w t c", t=2), stp[:])


def _build_nc(reps=1):
    """Build the kernel module. reps>1 wraps the whole computation in a
    hardware loop that repeats it (identical work each trip) -- used only to
    measure device execution time above the dispatch-noise floor."""
    import contextlib

    import concourse.tile as tile
    from concourse import bacc, mybir
    from concourse.masks import make_identity

    f32 = mybir.dt.float32
    f16 = mybir.dt.float16

    nc = bacc.Bacc("TRN2", target_bir_lowering=False, debug=False, num_devices=B)
    # x arrives pre-transposed (C, W) per row, fp16; W pre-arranged
    # (Cin, K, Cout) fp16 (host-side layout prep)
    x_d = nc.dram_tensor("x", [H, C, W_DIM], f16, kind="ExternalInput").ap()
    w_d = nc.dram_tensor("w", [C, K, C], f16, kind="ExternalInput").ap()
    o_d = nc.dram_tensor("out", [H, W_DIM, C], f32, kind="ExternalOutput").ap()

    with tile.TileContext(nc) as tc:
        with (
            tc.tile_pool(name="xin", bufs=6) as xin_pool,
            tc.tile_pool(name="state", bufs=1) as state_pool,
            tc.tile_pool(name="stage", bufs=4) as stage_pool,
            tc.tile_pool(name="const", bufs=1) as const_pool,
            tc.tile_pool(name="pconv", bufs=2, space="PSUM") as pconv_pool,
            tc.tile_pool(name="pout", bufs=2, space="PSUM") as pout_pool,
        ):
            # weights -> SBUF (ci partitions, K, co) fp16, single clean DMA
            # on the ACT queue (SP is busy with the x prefetches)
            w16 = const_pool.tile([P, K, C], f16, name="w16")
            nc.scalar.dma_start(w16[:], w_d)

            ident = const_pool.tile([P, P], f32, name="ident")
            make_identity(nc, ident[:])
            ident16 = const_pool.tile([P, P], f16, name="ident16")
            nc.vector.tensor_copy(ident16[:], ident[:])

            pools = (xin_pool, state_pool, stage_pool, pconv_pool, pout_pool)
            rep_ctx = tc.For_i(0, reps, 1) if reps > 1 else contextlib.nullcontext()
            with rep_ctx:
                _emit_body(nc, mybir, f32, f16, x_d, o_d, pools, w16, ident16)

    nc.compile()
    return nc


def _get_nc():
    if "nc" not in _NC_CACHE:
        _NC_CACHE["nc"] = _build_nc()
    return _NC_CACHE["nc"]


def _prep_x(xb):
    # (H, W, C) fp32 -> (H, C, W) fp16 host-side layout prep
    return np.ascontiguousarray(xb.transpose(0, 2, 1)).astype(np.float16)


def _prep_w(W):
    # (K, Cin, Cout) fp32 -> (Cin, K, Cout) fp16 host-side layout prep
    return np.ascontiguousarray(W.transpose(1, 0, 2)).astype(np.float16)


def kernel(x, W):
    """Full-input entry point: shard batch B across the 8 NeuronCores (data
    parallel), run the Bass kernel, gather per-core outputs."""
    from concourse.bass_utils import run_bass_kernel_spmd

    x = np.asarray(x, dtype=np.float32)
    W = np.asarray(W, dtype=np.float32)
    assert x.shape == (B, H, W_DIM, C), x.shape
    assert W.shape == (K, C, C), W.shape

    nc = _get_nc()
    w16 = _prep_w(W)
    in_maps = [{"x": _prep_x(x[b]), "w": w16} for b in range(B)]
    res = run_bass_kernel_spmd(nc, in_maps, core_ids=list(range(B)))
    return np.stack([np.asarray(res.results[b]["out"]) for b in range(B)], axis=0)
